# revision 1
# baseline (speedup 1.0000x reference)
"""Trainium2 Bass kernel for nn_ContrastiveLoss (N=384, D=128, 8 cores).

Math restructure (validated exactly against the reference):
  For each anchor row i and positive p (both off-diagonal), with
    a[i,j] = |y_i - y_j|,  w[i,j] = exp(-dist(z_i,z_j)/TEMP) * sigmoid(TAU*a[i,j]),
    u = w * [y_j > y_i] * [j != i],  v = w * [y_j <= y_i] * [j != i],
    S1[i,p] = sum_j u[i,j] * [a[i,j] < a[i,p]],  S0 likewise with v,
    T1 = sum_j u,  T0 = sum_j v:
  denom[i,p] = (POS_W-1)*S1 - NEG_W*S0 + NEG_W*T0 + T1
  loss = -(sum_{i,p!=i} s[i,p] - sum_{i,p!=i} log denom[i,p]) / (N*(N-1)),
  s = -dist/TEMP.  (The reference's row-max shift is exactly 0, so it's skipped.)

Per core (48 rows): the comparison tile C'[j,p] = [a_j < a_p] is built on the
Vector engine (one tensor_scalar is_gt per 128-j chunk) and contracted on the
TensorEngine with lhsT = [u_col, v_col] (M=2), accumulating S1/S0 in PSUM.
"""

import os
import sys

import numpy as np

for _p in ("/opt/trn_rl_repo", "/root/.axon_site/_ro/trn_rl_repo"):
    if os.path.isdir(_p) and _p not in sys.path:
        sys.path.insert(0, _p)

import concourse.bass as bass
import concourse.bacc as bacc
import concourse.mybir as mybir
from concourse import tile
from concourse.bass_utils import run_bass_kernel_spmd

F32 = mybir.dt.float32
AF = mybir.ActivationFunctionType
OP = mybir.AluOpType

B = 192          # batch
N = 2 * B        # 384 rows/cols of the pairwise matrices
D = 128          # embedding dim
NC = 8           # cores
R = N // NC      # 48 rows per core
CH = N // 128    # 3 chunks of the j dimension
PW = 920         # packed input width (919 used + 1 pad)

TEMP = 2.0
TAU = 1.0
POS_W = 0.1
NEG_W = 1.0


def _build_program():
    nc = bacc.Bacc("TRN2", target_bir_lowering=False, debug=False, num_devices=NC)

    # ---- I/O (f32). Everything arrives in ONE packed [128, PW] tensor so a
    # single DMA (one queue semaphore) feeds all consumers — walrus rejects
    # compute instructions carrying more than one DMA-queue sync wait.
    # Columns: 0:384 zT | 384:432 zTown | 432:480 yownrep | 480:528 ownidxrep
    #          528:531 ycolc | 531:534 jcolc | 534:918 yrep48 (rows 0:48)
    #          918:919 yowncol (rows 0:48)
    packed = nc.dram_tensor("packed", [128, PW], F32, kind="ExternalInput").ap()
    out = nc.dram_tensor("out", [2, R], F32, kind="ExternalOutput").ap()

    with tile.TileContext(nc) as tc:
        with (
            tc.tile_pool(name="big", bufs=1) as big,
            tc.tile_pool(name="small", bufs=1) as small,
            tc.tile_pool(name="chunk", bufs=3) as chunk,
            tc.tile_pool(name="arep", bufs=4) as arep_pool,
            tc.tile_pool(name="cmp", bufs=18) as cmp_pool,
            tc.tile_pool(name="ps_ss", bufs=1, space="PSUM") as ps_ss,
            tc.tile_pool(name="ps_pre", bufs=1, space="PSUM") as ps_pre,
            tc.tile_pool(name="ps_gt", bufs=3, space="PSUM") as ps_gt,
            tc.tile_pool(name="ps_acc", bufs=1, space="PSUM") as ps_acc,
            tc.tile_pool(name="ps_arep", bufs=2, space="PSUM") as ps_arep,
            tc.tile_pool(name="dram", bufs=1, space="DRAM") as dram_pool,
        ):
            # ---------- load inputs (ONE DMA) ----------
            pk = big.tile([128, PW], F32, tag="packed")
            nc.sync.dma_start(pk[:], packed)
            zT_s = pk[:, 0:N]
            zTown_s = pk[:, N : N + R]
            yownrep = pk[:, N + R : N + 2 * R]
            ownidxrep = pk[:, N + 2 * R : N + 3 * R]
            ycolc = pk[:, N + 3 * R : N + 3 * R + CH]
            jcolc = pk[:, N + 3 * R + CH : N + 3 * R + 2 * CH]
            yrep48 = pk[0:R, N + 3 * R + 2 * CH : 2 * N + 3 * R + 2 * CH]
            yowncol_s = pk[0:R, 2 * N + 3 * R + 2 * CH : 2 * N + 3 * R + 2 * CH + 1]

            ones128 = small.tile([128, 1], F32, tag="ones128")
            nc.vector.memset(ones128[:], 1.0)
            onesrow = small.tile([1, 128], F32, tag="onesrow")
            nc.vector.memset(onesrow[:], 1.0)

            # ---------- A row-block: a[i, p] = |y_p - y_i|  (exact on 2^-23 grid)
            a48raw = big.tile([R, N], F32, tag="a48raw")
            nc.vector.tensor_tensor(
                a48raw[:], yrep48, yowncol_s.to_broadcast((R, N)), op=OP.subtract
            )
            a48 = big.tile([R, N], F32, tag="a48")
            nc.scalar.activation(a48[:], a48raw[:], AF.Abs)

            # ---------- squared norms ----------
            zsq = big.tile([D, N], F32, tag="zsq")
            nc.vector.tensor_tensor(zsq[:], zT_s, zT_s, op=OP.mult)
            zsqown = small.tile([D, R], F32, tag="zsqown")
            nc.vector.tensor_tensor(zsqown[:], zTown_s, zTown_s, op=OP.mult)

            n2own_ps = ps_pre.tile([1, R], F32, tag="pre")
            nc.tensor.matmul(n2own_ps[:], ones128[:], zsqown[:], start=True, stop=True)
            n2own_s = small.tile([1, R], F32, tag="n2own_s")
            nc.vector.tensor_copy(n2own_s[:], n2own_ps[:])
            n2ownrep_ps = ps_pre.tile([128, R], F32, tag="pre")
            nc.tensor.matmul(n2ownrep_ps[:], onesrow[:], n2own_s[:], start=True, stop=True)
            n2ownrep = small.tile([128, R], F32, tag="n2ownrep")
            nc.vector.tensor_copy(n2ownrep[:], n2ownrep_ps[:])

            n2colc = small.tile([128, CH], F32, tag="n2colc")
            for c in range(CH):
                n2c_ps = ps_pre.tile([128, 1], F32, tag="pre")
                nc.tensor.matmul(
                    n2c_ps[:],
                    zsq[:, c * 128 : (c + 1) * 128],
                    ones128[:],
                    start=True,
                    stop=True,
                )
                nc.vector.tensor_copy(n2colc[:, c : c + 1], n2c_ps[:])

            # ---------- transposed-side prep per chunk ----------
            atc = small.tile([128, CH * R], F32, tag="atc")       # |y_j - y_i|
            uvt = small.tile([128, CH * 2 * R], F32, tag="uvt")   # interleaved u,v cols
            cs_ps = ps_acc.tile([1, 2 * R], F32, tag="acc")        # [sum_j w_off | sum_j dist_off]
            for c in range(CH):
                csl = slice(c * R, (c + 1) * R)
                atcraw = chunk.tile([128, R], F32, tag="atcraw")
                nc.vector.tensor_tensor(
                    atcraw[:],
                    yownrep,
                    ycolc[:, c : c + 1].to_broadcast((128, R)),
                    op=OP.subtract,
                )
                nc.scalar.activation(atc[:, csl], atcraw[:], AF.Abs)

                samet = chunk.tile([128, R], F32, tag="samet")
                nc.vector.tensor_tensor(
                    samet[:],
                    yownrep,
                    ycolc[:, c : c + 1].to_broadcast((128, R)),
                    op=OP.is_lt,
                )
                ndt = chunk.tile([128, R], F32, tag="ndt")
                nc.vector.tensor_tensor(
                    ndt[:],
                    ownidxrep,
                    jcolc[:, c : c + 1].to_broadcast((128, R)),
                    op=OP.not_equal,
                )

                gt_ps = ps_gt.tile([128, R], F32, tag="gt")
                nc.tensor.matmul(
                    gt_ps[:],
                    zT_s[:, c * 128 : (c + 1) * 128],
                    zTown_s,
                    start=True,
                    stop=True,
                )
                sqt = chunk.tile([128, R], F32, tag="sqt")
                # sq = n2own + n2col - 2*G
                nc.vector.tensor_scalar(sqt[:], gt_ps[:], -2.0, None, op0=OP.mult)
                nc.vector.tensor_tensor(sqt[:], sqt[:], n2ownrep[:], op=OP.add)
                nc.vector.tensor_tensor(
                    sqt[:], sqt[:], n2colc[:, c : c + 1].to_broadcast((128, R)), op=OP.add
                )
                sqr = chunk.tile([128, R], F32, tag="sqr")
                nc.scalar.activation(sqr[:], sqt[:], AF.Relu)
                distt = chunk.tile([128, R], F32, tag="distt")
                nc.scalar.activation(distt[:], sqr[:], AF.Sqrt)
                et = chunk.tile([128, R], F32, tag="et")
                nc.scalar.activation(et[:], distt[:], AF.Exp, scale=-1.0 / TEMP)
                dwt = chunk.tile([128, R], F32, tag="dwt")
                nc.scalar.activation(dwt[:], atc[:, csl], AF.Sigmoid, scale=TAU)

                # wd = [w*offdiag | dist*offdiag]  (one tile so one PE colsum matmul)
                wd = chunk.tile([128, 2 * R], F32, tag="wd")
                wt = chunk.tile([128, R], F32, tag="wt")
                nc.vector.tensor_tensor(wt[:], et[:], dwt[:], op=OP.mult)
                nc.vector.tensor_tensor(wd[:, 0:R], wt[:], ndt[:], op=OP.mult)
                nc.vector.tensor_tensor(wd[:, R : 2 * R], distt[:], ndt[:], op=OP.mult)

                # interleaved u,v columns for the main-loop lhsT
                base = c * 2 * R
                uv_u = uvt[:, base : base + 2 * R : 2]
                uv_v = uvt[:, base + 1 : base + 2 * R : 2]
                nc.vector.tensor_tensor(uv_u, wd[:, 0:R], samet[:], op=OP.mult)
                nc.vector.tensor_tensor(uv_v, wd[:, 0:R], uv_u, op=OP.subtract)

                nc.tensor.matmul(
                    cs_ps[:], ones128[:], wd[:], start=(c == 0), stop=(c == CH - 1)
                )

            cs_s = small.tile([1, 2 * R], F32, tag="cs_s")
            nc.vector.tensor_copy(cs_s[:], cs_ps[:])
            # cs_s[0, 0:R] = c_i = T0+T1 ;  cs_s[0, R:2R] = sum_{p!=i} dist[i,p]
            crep_ps = ps_pre.tile([128, R], F32, tag="pre")
            nc.tensor.matmul(crep_ps[:], onesrow[:], cs_s[0:1, 0:R], start=True, stop=True)
            crep48 = small.tile([128, R], F32, tag="crep48")
            nc.vector.tensor_copy(crep48[:], crep_ps[:])

            # ---------- main loop ----------
            # a48 rows flattened into partition 0 so the per-row PE outer
            # product (ones ⊗ a-row) can read its rhs at partition base 0.
            arowflat = small.tile([1, R * N], F32, tag="arowflat")
            nc.sync.dma_start(
                arowflat[0:1, :].rearrange("a (p f) -> a p f", p=R, f=N), a48[:]
            )
            # Transposed outputs: for row i, chunk-of-p psub, S1/S0 land in
            # sst[:, psub*2R + 2i + {0,1}] (partition = p within psub).
            sst_ps = ps_ss.tile([128, CH * 2 * R], F32, tag="sst")
            for i in range(R):
                arep_ps = ps_arep.tile([128, N], F32, tag="arep_ps")
                nc.tensor.matmul(
                    arep_ps[:],
                    onesrow[:],
                    arowflat[0:1, i * N : (i + 1) * N],
                    start=True,
                    stop=True,
                )
                arep = arep_pool.tile([128, N], F32, tag="arep")
                nc.vector.tensor_copy(arep[:], arep_ps[:])
                for c in range(CH):
                    cp = cmp_pool.tile([128, N], F32, tag="cp")
                    nc.vector.tensor_scalar(
                        cp[:],
                        arep[:],
                        atc[:, c * R + i : c * R + i + 1],
                        None,
                        op0=OP.is_gt,
                    )
                    for ps in range(CH):
                        # One accumulation group spans the whole bank: only the
                        # very first matmul starts it (start=True pending-zeroes
                        # the full 2KB zero region); per-byte has_written bits
                        # make each sub-region's first write an overwrite.
                        nc.tensor.matmul(
                            sst_ps[:, ps * 2 * R + 2 * i : ps * 2 * R + 2 * i + 2],
                            cp[:, ps * 128 : (ps + 1) * 128],
                            uvt[:, c * 2 * R + 2 * i : c * 2 * R + 2 * i + 2],
                            start=(i == 0 and c == 0 and ps == 0),
                            stop=(i == R - 1 and c == CH - 1 and ps == CH - 1),
                            skip_group_check=True,
                        )
            sst = small.tile([128, CH * 2 * R], F32, tag="sst_sb")
            nc.vector.tensor_copy(sst[:], sst_ps[:])

            # ---------- postprocess (transposed layout) ----------
            # dent[p_local, ps*R+i] = den[i, ps*128+p_local]
            dent = small.tile([128, CH * R], F32, tag="dent")
            nc.vector.tensor_scalar(
                dent[:], sst[:, 0 : CH * 2 * R : 2], POS_W - 1.0, None, op0=OP.mult
            )
            nc.vector.tensor_tensor(
                dent[:], dent[:], sst[:, 1 : CH * 2 * R : 2], op=OP.subtract
            )
            for c in range(CH):
                nc.vector.tensor_tensor(
                    dent[:, c * R : (c + 1) * R],
                    dent[:, c * R : (c + 1) * R],
                    crep48[:],
                    op=OP.add,
                )
            lnt = small.tile([128, CH * R], F32, tag="lnt")
            nc.scalar.activation(lnt[:], dent[:], AF.Ln)
            lds_ps = ps_acc.tile([1, CH * R], F32, tag="acc")
            nc.tensor.matmul(lds_ps[:], ones128[:], lnt[:], start=True, stop=True)
            lds = small.tile([1, CH * R], F32, tag="lds_s")
            nc.vector.tensor_copy(lds[:], lds_ps[:])

            # combine psub partials; subtract ln(c_i) for the excluded p=i column
            lnc = small.tile([1, R], F32, tag="lnc")
            nc.scalar.activation(lnc[:], cs_s[0:1, 0:R], AF.Ln)
            lnc2 = small.tile([1, R], F32, tag="lnc2")
            nc.vector.tensor_copy(lnc2[:], lnc[:])
            acc = small.tile([1, R], F32, tag="acc")
            nc.vector.tensor_tensor(acc[:], lds[0:1, 0:R], lds[0:1, R : 2 * R], op=OP.add)
            nc.vector.tensor_tensor(acc[:], acc[:], lds[0:1, 2 * R : 3 * R], op=OP.add)
            logd_t = small.tile([1, R], F32, tag="logd_t")
            nc.vector.tensor_tensor(logd_t[:], acc[:], lnc2[:], op=OP.subtract)
            # row0 = sum_{p!=i} s[i,p] = -dist_off_rowsum / TEMP
            ssum_t = small.tile([1, R], F32, tag="ssum_t")
            nc.scalar.activation(
                ssum_t[:], cs_s[0:1, R : 2 * R], AF.Copy, scale=-1.0 / TEMP
            )
            nc.sync.dma_start(out[0:1, :], ssum_t[:])
            nc.sync.dma_start(out[1:2, :], logd_t[:])

    nc.compile()
    return nc


_NC_CACHE = None


def _get_nc():
    global _NC_CACHE
    if _NC_CACHE is None:
        _NC_CACHE = _build_program()
    return _NC_CACHE


def _make_in_maps(embeddings, targets):
    emb = np.ascontiguousarray(np.asarray(embeddings, dtype=np.float32))
    tgt = np.ascontiguousarray(np.asarray(targets, dtype=np.float32))
    z = emb.transpose(1, 0, 2).reshape(N, D)
    zT = np.ascontiguousarray(z.T)                       # [D, N]
    y = np.concatenate([tgt, tgt], axis=0)[:, 0]         # [N]
    jidx = np.arange(N, dtype=np.float32)
    in_maps = []
    for core in range(NC):
        sl = slice(core * R, (core + 1) * R)
        p = np.zeros((128, PW), np.float32)
        p[:, 0:N] = zT
        p[:, N : N + R] = zT[:, sl]
        p[:, N + R : N + 2 * R] = y[None, sl]                       # yownrep
        p[:, N + 2 * R : N + 3 * R] = jidx[None, sl]                # ownidxrep
        p[:, N + 3 * R : N + 3 * R + CH] = y.reshape(CH, 128).T     # ycolc
        p[:, N + 3 * R + CH : N + 3 * R + 2 * CH] = jidx.reshape(CH, 128).T
        p[0:R, N + 3 * R + 2 * CH : 2 * N + 3 * R + 2 * CH] = y[None, :]  # yrep48
        p[0:R, 2 * N + 3 * R + 2 * CH] = y[sl]                      # yowncol
        in_maps.append({"packed": p})
    return in_maps


def _reduce_outs(outs_list):
    tot_s = 0.0
    tot_logd = 0.0
    for o in outs_list:
        o = np.asarray(o, dtype=np.float64)
        tot_s += o[0, :].sum()
        tot_logd += o[1, :].sum()
    loss = -(tot_s - tot_logd) / (N * (N - 1))
    return np.float32(loss)


def _run(embeddings, targets, trace=False, **kw):
    nc = _get_nc()
    in_maps = _make_in_maps(embeddings, targets)
    res = run_bass_kernel_spmd(nc, in_maps, list(range(NC)), trace=trace, **kw)
    outs = [res.results[c]["out"] for c in range(NC)]
    return _reduce_outs(outs), res


def kernel(embeddings, targets):
    loss, _ = _run(embeddings, targets, trace=False)
    return loss



# revision 5
# speedup vs baseline: 1.5963x; 1.5963x over previous
"""Trainium2 Bass kernel for nn_ContrastiveLoss (N=384, D=128, 8 cores).

Sorted-domain prefix-sum formulation (validated vs the reference):
  Sort columns by label value y once (host-side packing).  For row i at
  sorted position m, the contrastive mask sums collapse to interval sums
  of the sorted weight rows:
    U[m,k] = w[m,k]*[ys_k > ys_m],  V[m,k] = w[m,k]*[ys_k <= ys_m][k != m]
    PUex[m,t] = sum_{k<t} U[m,k],   PVex likewise (exclusive prefixes)
    denom[m,p] = (POS_W-1)*PUex[m, t1[m,p]] + T1[m] + NEG_W*PVex[m, t0[m,p]]
  where the rank tables t1/t0 depend only on the (tiny) targets input and
  are precomputed host-side.  The N^3 masked-comparison einsum becomes:
  3 prefix matmuls (PE) + GPSIMD ap_gather lookups + one fused Ln+rowsum.

Per core (48 sorted rows): w-matrix via 3 distance matmuls + DVE/ACT ops,
prefix sums in one PSUM tile [96, 385], D = scaled prefix array [48, 770],
replicated x16 across partitions by DMA so each GPSIMD 16-lane group
gathers one row's 784 indices; Ln(+accumulate) on ACT, final column-sum
matmul.  Host sums the per-core partials.
"""

import os
import sys

import numpy as np

for _p in ("/opt/trn_rl_repo", "/root/.axon_site/_ro/trn_rl_repo"):
    if os.path.isdir(_p) and _p not in sys.path:
        sys.path.insert(0, _p)

import concourse.bass as bass
import concourse.bacc as bacc
import concourse.mybir as mybir
from concourse import tile
from concourse.bass_utils import run_bass_kernel_spmd

F32 = mybir.dt.float32
I16 = mybir.dt.int16
AF = mybir.ActivationFunctionType
OP = mybir.AluOpType

B = 192          # batch
N = 2 * B        # 384 rows/cols
D = 128          # embedding dim
NC = 8           # cores
R = N // NC      # 48 rows per core
CH = N // 128    # 3 chunks of the k dimension
NB = R // 8      # 6 gather blocks of 8 rows
TW = N + 1       # 385 prefix positions t in 0..384
DW = 2 * TW      # 770 = [DPU | DPV]
GW = 784         # gather indices per row (mult of 16): 392 + 392
HGW = GW // 2    # 392 = 384 p-cols + 8 own-copies
IW = GW // 16    # 49 idx columns per block

TEMP = 2.0
TAU = 1.0
POS_W = 0.1
NEG_W = 1.0

# packed fp32 input layout [128, PW]
O_ZT = 0                 # 0:384      zsT (sorted z, transposed)
O_ZOWN = N               # 384:432    zsT own columns
O_YOWN = N + R           # 432:480    ys of own rows (bcast down partitions)
O_IOWN = N + 2 * R       # 480:528    global sorted idx of own rows (f32)
O_YCOL = N + 3 * R       # 528:531    ys per k-chunk column
O_JCOL = O_YCOL + CH     # 531:534    global k idx per chunk column (f32)
O_IOTA = O_JCOL + CH     # 534:919    iota row 0..384 (partition 0)
O_SEL = O_IOTA + TW      # 919:920    sel16 column (1.0 at part%16==0)
PW = O_SEL + 1           # 920
# idx input: int16 [128, NB*IW]


def _build_program():
    nc = bacc.Bacc("TRN2", target_bir_lowering=False, debug=False, num_devices=NC)

    packed = nc.dram_tensor("packed", [128, PW], F32, kind="ExternalInput").ap()
    idxs = nc.dram_tensor("idxs", [128, NB * IW], I16, kind="ExternalInput").ap()
    out = nc.dram_tensor("out", [2, R], F32, kind="ExternalOutput").ap()

    with tile.TileContext(nc) as tc:
        with (
            tc.tile_pool(name="big", bufs=1) as big,
            tc.tile_pool(name="small", bufs=1) as small,
            tc.tile_pool(name="chunk", bufs=3) as chunk,
            tc.tile_pool(name="ps_a", bufs=2, space="PSUM") as ps_a,
            tc.tile_pool(name="ps_gt", bufs=3, space="PSUM") as ps_gt,
            tc.tile_pool(name="ps_uv", bufs=1, space="PSUM") as ps_uv,
            tc.tile_pool(name="ps_cs", bufs=1, space="PSUM") as ps_cs,
        ):
            # ---------- input DMAs ----------
            pk = big.tile([128, PW], F32, tag="pk")
            nc.sync.dma_start(pk[:], packed)
            zT = pk[:, O_ZT:O_ZT + N]
            zTown = pk[:, O_ZOWN:O_ZOWN + R]
            ysown = pk[:, O_YOWN:O_YOWN + R]
            idxown = pk[:, O_IOWN:O_IOWN + R]
            yscol = pk[:, O_YCOL:O_YCOL + CH]
            jcol = pk[:, O_JCOL:O_JCOL + CH]
            iotarow = pk[0:1, O_IOTA:O_IOTA + TW]
            sel16 = pk[:, O_SEL:O_SEL + 1]

            it0 = big.tile([128, NB * IW], I16, tag="it0")
            nc.sync.dma_start(it0[:], idxs)
            # route idx through DVE so gathers carry only one DMA-queue wait
            it = big.tile([128, NB * IW], I16, tag="it")
            nc.vector.tensor_copy(it[:], it0[:])

            ones128 = small.tile([128, 1], F32, tag="ones128")
            nc.vector.memset(ones128[:], 1.0)
            onesrow = small.tile([1, 128], F32, tag="onesrow")
            nc.vector.memset(onesrow[:], 1.0)

            # ---------- squared norms ----------
            zsq = big.tile([128, N], F32, tag="zsq")
            nc.vector.tensor_tensor(zsq[:], zT, zT, op=OP.mult)
            zsqown = small.tile([128, R], F32, tag="zsqown")
            nc.vector.tensor_tensor(zsqown[:], zTown, zTown, op=OP.mult)

            n2own_ps = ps_a.tile([1, R], F32, tag="a")
            nc.tensor.matmul(n2own_ps[:], ones128[:], zsqown[:], start=True, stop=True)
            n2own_s = small.tile([1, R], F32, tag="n2own_s")
            nc.vector.tensor_copy(n2own_s[:], n2own_ps[:])
            n2ownrep_ps = ps_a.tile([128, R], F32, tag="a")
            nc.tensor.matmul(n2ownrep_ps[:], onesrow[:], n2own_s[:], start=True, stop=True)
            n2ownrep = small.tile([128, R], F32, tag="n2ownrep")
            nc.vector.tensor_copy(n2ownrep[:], n2ownrep_ps[:])

            n2colc = small.tile([128, CH], F32, tag="n2colc")
            for c in range(CH):
                n2c_ps = ps_a.tile([128, 1], F32, tag="a")
                nc.tensor.matmul(
                    n2c_ps[:], zsq[:, c * 128:(c + 1) * 128], ones128[:],
                    start=True, stop=True,
                )
                nc.vector.tensor_copy(n2colc[:, c:c + 1], n2c_ps[:])

            # ---------- Texc: [k < t] per chunk ----------
            trep_ps = ps_a.tile([128, TW], F32, tag="a")
            nc.tensor.matmul(trep_ps[:], onesrow[:], iotarow, start=True, stop=True)
            trep = big.tile([128, TW], F32, tag="trep")
            nc.vector.tensor_copy(trep[:], trep_ps[:])
            texc = big.tile([128, CH * TW], F32, tag="texc")
            for c in range(CH):
                nc.vector.tensor_scalar(
                    texc[:, c * TW:(c + 1) * TW], trep[:], jcol[:, c:c + 1], None,
                    op0=OP.is_gt,
                )

            # ---------- stage A: w matrix (transposed) per chunk ----------
            UW = 112  # per-chunk lhsT cols: U(48) | zeros(16) | V(48)
            uvt = big.tile([128, CH * UW], F32, tag="uvt")
            nc.vector.memset(uvt[:], 0.0)
            cs_ps = ps_cs.tile([1, R], F32, tag="cs")
            for c in range(CH):
                ycolbc = yscol[:, c:c + 1].to_broadcast((128, R))
                samet = chunk.tile([128, R], F32, tag="samet")
                nc.vector.tensor_tensor(samet[:], ysown, ycolbc, op=OP.is_lt)
                ndt = chunk.tile([128, R], F32, tag="ndt")
                nc.vector.tensor_tensor(
                    ndt[:], idxown, jcol[:, c:c + 1].to_broadcast((128, R)),
                    op=OP.not_equal,
                )
                gt_ps = ps_gt.tile([128, R], F32, tag="gt")
                nc.tensor.matmul(
                    gt_ps[:], zT[:, c * 128:(c + 1) * 128], zTown,
                    start=True, stop=True,
                )
                sqt = chunk.tile([128, R], F32, tag="sqt")
                nc.vector.scalar_tensor_tensor(
                    sqt[:], gt_ps[:], -2.0, n2ownrep[:], op0=OP.mult, op1=OP.add
                )
                sqr = chunk.tile([128, R], F32, tag="sqr")
                nc.scalar.activation(sqr[:], sqt[:], AF.Relu, bias=n2colc[:, c:c + 1])
                distt = chunk.tile([128, R], F32, tag="distt")
                nc.scalar.activation(distt[:], sqr[:], AF.Sqrt)
                et = chunk.tile([128, R], F32, tag="et")
                nc.scalar.activation(et[:], distt[:], AF.Exp, scale=-1.0 / TEMP)
                atcraw = chunk.tile([128, R], F32, tag="atcraw")
                nc.vector.tensor_tensor(atcraw[:], ysown, ycolbc, op=OP.subtract)
                atc = chunk.tile([128, R], F32, tag="atc")
                nc.scalar.activation(atc[:], atcraw[:], AF.Abs)
                dwt = chunk.tile([128, R], F32, tag="dwt")
                nc.scalar.activation(dwt[:], atc[:], AF.Sigmoid, scale=TAU)
                wt = chunk.tile([128, R], F32, tag="wt")
                nc.vector.tensor_tensor(wt[:], et[:], dwt[:], op=OP.mult)
                # U / V columns for the prefix matmul lhsT
                nc.vector.tensor_tensor(
                    uvt[:, c * UW:c * UW + R], wt[:], samet[:], op=OP.mult
                )
                vm = chunk.tile([128, R], F32, tag="vm")
                nc.vector.tensor_tensor(vm[:], ndt[:], samet[:], op=OP.subtract)
                nc.vector.tensor_tensor(
                    uvt[:, c * UW + 64:c * UW + 64 + R], wt[:], vm[:], op=OP.mult
                )
                # off-diagonal dist row-sums (for the s term)
                wdist = chunk.tile([128, R], F32, tag="wdist")
                nc.vector.tensor_tensor(wdist[:], distt[:], ndt[:], op=OP.mult)
                nc.tensor.matmul(
                    cs_ps[:], ones128[:], wdist[:], start=(c == 0), stop=(c == CH - 1)
                )

            # ---------- prefix sums: puv[0:48]=PUex rows, [48:96]=PVex ----------
            puv_ps = ps_uv.tile([112, TW], F32, tag="puv")
            for c in range(CH):
                nc.tensor.matmul(
                    puv_ps[:], uvt[:, c * UW:(c + 1) * UW],
                    texc[:, c * TW:(c + 1) * TW],
                    start=(c == 0), stop=(c == CH - 1),
                )

            # ---------- D = [(POS_W-1)*PUex + T1 | NEG_W*PVex]  [48, 770] ----------
            t1sb = small.tile([R, 1], F32, tag="t1sb")
            nc.vector.tensor_copy(t1sb[:], puv_ps[0:R, N:N + 1])
            darr = big.tile([R, DW], F32, tag="darr")
            nc.vector.scalar_tensor_tensor(
                darr[:, 0:TW], puv_ps[0:R, :], POS_W - 1.0,
                t1sb[:].to_broadcast((R, TW)), op0=OP.mult, op1=OP.add,
            )
            pvs = small.tile([112, TW], F32, tag="pvs")
            if NEG_W == 1.0:
                nc.vector.tensor_copy(pvs[64:112, :], puv_ps[64:112, :])
            else:
                nc.vector.tensor_scalar(
                    pvs[64:112, :], puv_ps[64:112, :], NEG_W, None, op0=OP.mult
                )
            # shift PVex rows from partitions 64..111 to 0..47 (cols 385:770)
            nc.sync.dma_start(darr[:, TW:DW], pvs[64:112, :])

            # ---------- per block: replicate D x16, gather, ln+accumulate ----------
            crossin = big.tile([128, NB * DW], F32, tag="crossin")
            gout = big.tile([128, NB * GW], F32, tag="gout")
            rowtots = small.tile([128, NB], F32, tag="rowtots")
            for b in range(NB):
                cin_b = crossin[:, b * DW:(b + 1) * DW]
                v = cin_b.rearrange("(r g) f -> g r f", g=16)
                for g0 in range(16):
                    nc.sync.dma_start(v[g0], darr[b * 8:(b + 1) * 8, :])
                go_b = gout[:, b * GW:(b + 1) * GW]
                nc.gpsimd.ap_gather(
                    go_b, cin_b, it[:, b * IW:(b + 1) * IW],
                    channels=128, num_elems=DW, d=1, num_idxs=GW,
                )
                dn = chunk.tile([128, HGW], F32, tag="dn")
                nc.vector.tensor_tensor(
                    dn[:], go_b[:, 0:HGW], go_b[:, HGW:GW], op=OP.add
                )
                lnt = chunk.tile([128, HGW], F32, tag="lnt")
                acc = chunk.tile([128, 1], F32, tag="acc")
                nc.scalar.activation(lnt[:], dn[:], AF.Ln, accum_out=acc[:])
                # accum counts own-col once in 0..383 plus 8 pad copies -> -9x
                nc.vector.scalar_tensor_tensor(
                    rowtots[:, b:b + 1], lnt[:, N:N + 1], -9.0, acc[:],
                    op0=OP.mult, op1=OP.add,
                )

            # ---------- final reduction ----------
            lnacc_ps = ps_a.tile([1, NB], F32, tag="a")
            nc.tensor.matmul(lnacc_ps[:], sel16, rowtots[:], start=True, stop=True)
            outrow = small.tile([1, R + NB], F32, tag="outrow")
            nc.vector.tensor_copy(outrow[0:1, 0:R], cs_ps[:])
            nc.vector.tensor_copy(outrow[0:1, R:R + NB], lnacc_ps[:])
            nc.sync.dma_start(out[0:1, :], outrow[0:1, 0:R])
            nc.sync.dma_start(out[1:2, 0:NB], outrow[0:1, R:R + NB])

    nc.compile()
    return nc


_NC_CACHE = None


def _get_nc():
    global _NC_CACHE
    if _NC_CACHE is None:
        _NC_CACHE = _build_program()
    return _NC_CACHE


def _host_prep(embeddings, targets):
    emb = np.ascontiguousarray(np.asarray(embeddings, dtype=np.float32))
    tgt = np.ascontiguousarray(np.asarray(targets, dtype=np.float32))
    z = emb.transpose(1, 0, 2).reshape(N, D)
    y = np.concatenate([tgt, tgt], axis=0)[:, 0]

    order = np.argsort(y, kind="stable")
    ys = y[order]
    zs = z[order]
    zsT = np.ascontiguousarray(zs.T)  # [D, N]

    # rank tables (depend only on targets)
    A = np.abs(ys[None, :] - ys[:, None]).astype(np.float32)
    hi = np.searchsorted(ys, ys, side="right") - 1
    t1 = np.empty((N, N), np.int32)
    t0 = np.empty((N, N), np.int32)
    for m in range(N):
        h = hi[m]
        q1 = np.searchsorted(A[m, h + 1:], A[m], side="left")
        q0 = np.searchsorted(A[m, :h + 1][::-1], A[m], side="left")
        t1[m] = h + 1 + q1
        t0[m] = h + 1 - q0

    jidx = np.arange(N, dtype=np.float32)
    iota = np.arange(TW, dtype=np.float32)
    sel = (np.arange(128) % 16 == 0).astype(np.float32)

    in_maps = []
    for core in range(NC):
        sl = slice(core * R, (core + 1) * R)
        p = np.zeros((128, PW), np.float32)
        p[:, O_ZT:O_ZT + N] = zsT
        p[:, O_ZOWN:O_ZOWN + R] = zsT[:, sl]
        p[:, O_YOWN:O_YOWN + R] = ys[None, sl]
        p[:, O_IOWN:O_IOWN + R] = jidx[None, sl]
        p[:, O_YCOL:O_YCOL + CH] = ys.reshape(CH, 128).T
        p[:, O_JCOL:O_JCOL + CH] = jidx.reshape(CH, 128).T
        p[0, O_IOTA:O_IOTA + TW] = iota
        p[:, O_SEL] = sel

        # gather index tile: per block b (8 rows), wrapped 16-partition layout
        it = np.zeros((128, NB * IW), np.int16)
        for bidx in range(NB):
            rows = core * R + bidx * 8 + np.arange(8)
            q = np.empty((8, GW), np.int16)
            q[:, 0:N] = t1[rows]
            q[:, N:HGW] = (hi[rows] + 1)[:, None]
            q[:, HGW:HGW + N] = TW + t0[rows]
            q[:, HGW + N:GW] = (TW + hi[rows] + 1)[:, None]
            # position j of group g -> it[16g + j%16, IW*b + j//16]
            wrapped = q.reshape(8, IW, 16).transpose(0, 2, 1)  # [g, j%16, j//16]
            it[:, bidx * IW:(bidx + 1) * IW] = wrapped.reshape(128, IW)
        in_maps.append({"packed": p, "idxs": it})
    return in_maps


def _reduce_outs(outs_list):
    tot_dist = 0.0
    tot_logd = 0.0
    for o in outs_list:
        o = np.asarray(o, dtype=np.float64)
        tot_dist += o[0, :].sum()
        tot_logd += o[1, 0:NB].sum()
    s_total = -tot_dist / TEMP
    loss = -(s_total - tot_logd) / (N * (N - 1))
    return np.float32(loss)


def _run(embeddings, targets, trace=False, **kw):
    nc = _get_nc()
    in_maps = _host_prep(embeddings, targets)
    res = run_bass_kernel_spmd(nc, in_maps, list(range(NC)), trace=trace, **kw)
    outs = [res.results[c]["out"] for c in range(NC)]
    return _reduce_outs(outs), res


def kernel(embeddings, targets):
    loss, _ = _run(embeddings, targets, trace=False)
    return loss
